# revision 1
# baseline (speedup 1.0000x reference)
"""Trainium2 Bass kernel for nn_MeanStdStiffRegularizer (segment reduce).

Strategy (8 NeuronCores, segment-bucketed data parallel):
  - The host groups edges by segment id (stable counting sort) and packs
    them into a fixed-capacity padded layout: every segment owns one
    column slot in each of ROUNDS*[128, 512] blocks per core, so column
    index == segment id and partition index == edge slot.  Pad slots
    hold x = 1.0 (log(|1|+eps) ~ 0, square ~ 0 -> pads only bias the
    x-sum by exactly the pad count, which the host subtracts).
  - x ships as fp8 e5m2 (verified: 1.1e-3 rel err on the final loss,
    vs 2e-2 tolerance; e4m3 fails - its 2^-9 subnormal floor distorts
    log of small |x|).  That halves the input-DMA, which would otherwise
    pace the whole pipeline.
  - log(|x|) via the Mitchell bit trick (verified 4.4e-3 rel err):
    for e5m2, u = bits & 0x7f gives ln|x| ~ ln2/4 * u - 15*ln2; the
    affine part and its square expand into exact HOST algebra over
    per-segment sums of u and u^2.  So the device only extracts bytes
    (DVE int16 tensor_scalar shift/mask at 4x, writing f16 into
    parity-blocked [even256|odd256] round layouts), squares them (half
    on DVE 2x, half on ACT - the two engines split the former Ln cost),
    and runs the same ones-stationary FD=512 matmuls (psum column ==
    parity-permuted segment).  The last round's u ships pre-extracted
    from the host so the tail chain is ACT-only.
    Four PE column tiles each accumulate every 4th block; the 3 value
    streams use 3 PSUM banks.
  - Each core returns [4 tiles, 3 streams, 512 segments] partial sums;
    the host adds tiles/cores, subtracts pad contributions, divides by
    np.bincount counts, and finishes the tiny mean/std loss in float64.
"""

import sys
import types

import numpy as np

N_EDGES = 16777216
NUM_SEG = 512
STRENGTH = 0.01
STD_WEIGHT = 0.5
EPS = 1e-6

N_CORES = 8
P = 128
ROUNDS = 33          # per-core [128, 512] blocks; capacity/segment = 8*33*128
N_PETILE = 4         # PE column tiles (each 32 stationary cols of ones)


def _dma_chunks(rounds):
    """Input-DMA chunk sizes (rounds): exponential lead-in so the first
    compute starts after a tiny transfer; each chunk is ONE DMA
    instruction (descriptor generation on the Sync queue costs ~0.6us
    each) sized so completion stays well ahead of the ACT pipeline."""
    sizes = []
    rem = rounds
    for t in (1, 2, 4, 8, 8):
        if rem - t < 10:
            break
        sizes.append(t)
        rem -= t
    sizes.append(rem)
    assert sum(sizes) == rounds
    return sizes


def _taper(rem):
    """Descending piece sizes (<=8) ending ...,3,2 so each DVE square
    hides under the next piece's Ln."""
    if rem <= 0:
        return []
    if rem <= 2:
        return [rem]
    if rem <= 5:
        return [rem - 2, 2]
    if rem <= 9:
        return [rem - 5, 3, 2]
    out = []
    while rem > 9:
        out.append(min(8, rem - 9))
        rem -= out[-1]
    return out + _taper(rem)


def _subops(rounds):
    """(chunk, round0, nrounds) compute pieces: <=8 rounds per op, with a
    tapered ending (final xa round excluded); pieces never span chunks."""
    chunks = _dma_chunks(rounds)
    out = []
    r0 = 0
    for ci, rc in enumerate(chunks):
        last_chunk = ci == len(chunks) - 1
        pieces = _taper(rc - 1) if last_chunk else _taper(rc) if False else None
        if last_chunk:
            pieces = _taper(rc - 1)
        else:
            pieces = []
            rem = rc
            while rem > 0:
                pieces.append(min(8, rem))
                rem -= pieces[-1]
        rr = r0
        for t in pieces:
            out.append((ci, rr, t))
            rr += t
        r0 += rc
    assert all(n <= 8 for _, _, n in out)
    assert sum(n for _, _, n in out) == rounds - 1
    return chunks, out


def _install_ntff_hook():
    """Register the axon NTFF profiling hook (missing antenv.axon_hooks)."""
    if "antenv.axon_hooks" in sys.modules:
        return
    mod = types.ModuleType("antenv.axon_hooks")
    _h = [None]
    mod.set_axon_ntff_profile_hook = lambda h: _h.__setitem__(0, h)
    mod.get_axon_ntff_profile_hook = lambda: _h[0]
    sys.modules["antenv.axon_hooks"] = mod
    try:
        from trn_agent_boot.trn_boot import _ntff_profile_via_ctypes

        mod.set_axon_ntff_profile_hook(
            _ntff_profile_via_ctypes("/opt/axon/libaxon_pjrt.so")
        )
    except Exception:
        pass


_NO_SPLIT_OPCODES = {
    "CollectiveCompute",
}


def _split_sync_waits(bir_json_bytes):
    """Rewrite BIR so no TPB instruction carries more than one sync wait.

    The walrus codegen in this container supports a single sync-wait slot
    per TPB instruction ("Too many sync wait commands" otherwise).  Extra
    waits are hoisted onto EventSemaphore instructions inserted immediately
    before, on the same engine (same issue-gating semantics).
    """
    import json

    j = json.loads(bir_json_bytes)
    n_split = 0
    uid = [0]
    for f in j["functions"]:
        for b in f["blocks"]:
            out = []
            for ins in b["instructions"]:
                si = ins.get("sync_info")
                ow = (si or {}).get("on_wait") or []
                if len(ow) > 1 and ins.get("opcode") not in _NO_SPLIT_OPCODES:
                    for w in ow[:-1]:
                        uid[0] += 1
                        out.append(
                            {
                                "debug": ins.get("debug", 0),
                                "engine": ins["engine"],
                                "ins": [],
                                "name": f"{ins['name']}-wsplit{uid[0]}",
                                "opcode": "EventSemaphore",
                                "outs": [],
                                "sync_info": {"on_update": [], "on_wait": [w]},
                            }
                        )
                    si["on_wait"] = [ow[-1]]
                    n_split += 1
                out.append(ins)
            b["instructions"] = out
    return json.dumps(j).encode(), n_split


def build_nc(rounds=ROUNDS, n_cores=N_CORES):
    """Build the per-core Bass program (SPMD: same program on every core)."""
    import concourse.bass as bass
    import concourse.tile as tile
    from concourse import mybir

    f32 = mybir.dt.float32
    bf16 = mybir.dt.bfloat16
    i16 = mybir.dt.int16
    AOP = mybir.AluOpType
    ACT = mybir.ActivationFunctionType

    cols = rounds * NUM_SEG
    nc = bass.Bass(
        "TRN2", target_bir_lowering=False, debug=False, num_devices=n_cores
    )
    f8 = mybir.dt.float8e5
    x_d = nc.dram_tensor("x", [P, cols], f8, kind="ExternalInput")
    f16 = mybir.dt.float16
    ul_d = nc.dram_tensor("ul", [P, NUM_SEG], f16, kind="ExternalInput")
    out_d = nc.dram_tensor(
        "out", [N_PETILE, 3, NUM_SEG], f32, kind="ExternalOutput"
    )

    chunks, subops = _subops(rounds)
    cstarts = []
    acc_r = 0
    for rc in chunks:
        cstarts.append(acc_r)
        acc_r += rc

    with tile.TileContext(nc) as tc:
        with (
            tc.tile_pool(name="const", bufs=1) as cpool,
            tc.tile_pool(name="io", bufs=1) as io,
            tc.tile_pool(name="mid", bufs=3) as mid,
            tc.tile_pool(name="fin", bufs=1) as fin,
            tc.tile_pool(name="acc", bufs=1, space="PSUM") as psum,
        ):
            ones = cpool.tile([P, 32], bf16)
            nc.vector.memset(ones[:], 1.0)
            ones8 = cpool.tile([P, 32], f8)
            nc.vector.memset(ones8[:], 1.0)
            eps_t = cpool.tile([P, 1], f32)
            nc.vector.memset(eps_t[:], EPS)

            # one DMA instruction per input chunk, all issued up front
            ctiles = []
            for ci, rc in enumerate(chunks):
                ct = io.tile([P, rc * NUM_SEG], f8, tag=f"c{ci}", name="ct")
                nc.sync.dma_start(ct[:], x_d[:, cstarts[ci] * NUM_SEG :
                                              (cstarts[ci] + rc) * NUM_SEG])
                ctiles.append(ct)
            ulast = io.tile([P, NUM_SEG], f16, tag="ulast", name="ulast")
            nc.sync.dma_start(ulast[:], ul_d[:, :])

            # PSUM: x+l partials share a 2-bank tile (fused final copy);
            # q gets its own bank.  Each PE column tile q writes rows
            # [32q, 32q+32) (identical rows: ones stationary).
            acc01 = psum.tile([P, 2, NUM_SEG], f32, tag="acc01", name="a01")
            acc2 = psum.tile([P, NUM_SEG], f32, tag="acc2", name="a2")
            # PE col tile for (round, stream): rotate so consecutive MMs
            # hit different array tiles AND different PSUM banks.
            tile_of = lambda r, j: (3 * r + j) % N_PETILE
            n_chain = {}
            for r in range(rounds):
                for j in range(3):
                    k = (j, tile_of(r, j))
                    n_chain[k] = n_chain.get(k, 0) + 1

            mm_done = {k: 0 for k in n_chain}

            def emit_mm(r, j, src, ss):
                q = tile_of(r, j)
                k = (j, q)
                out = (
                    acc01[q * 32 : (q + 1) * 32, j, :]
                    if j < 2
                    else acc2[q * 32 : (q + 1) * 32, :]
                )
                nc.tensor.matmul(
                    out,
                    (ones8 if j == 0 else ones)[:, :],
                    src[:, ss],
                    start=(mm_done[k] == 0),
                    stop=(mm_done[k] == n_chain[k] - 1),
                    tile_position=(0, q * 32),
                )
                mm_done[k] += 1

            outsb = fin.tile([P, 3, NUM_SEG], f32)
            for ci, r0, rm in subops:
                last_sub = (ci, r0, rm) == subops[-1]
                w = rm * NUM_SEG
                o0 = (r0 - cstarts[ci]) * NUM_SEG
                xt = ctiles[ci][:, o0 : o0 + w]

                # u = e5m2 exponent/mantissa bits of |x|: extract the lo
                # and hi bytes of each int16 pair on DVE (tensor_scalar
                # shift/mask, single-src -> 4x), writing f16 values into a
                # parity-blocked [even 256 | odd 256] per-round layout so
                # one FD=512 matmul per round still covers every segment.
                rmx = rm
                # byte-extract u (int16->int16 bitvec ops, 4x), then cast
                # to f16 via tensor_copy (the only op allowed to cast)
                ue = mid.tile([P, 8 * 256], i16, tag="ue", name="ue")[:, : w // 2]
                uh = mid.tile([P, 8 * 256], i16, tag="uh", name="uh")[:, : w // 2]
                xi = xt.bitcast(i16)
                nc.vector.tensor_scalar(ue, xi, 0x007F, None, AOP.bitwise_and)
                nc.vector.tensor_scalar(
                    uh, xi, 8, 0x7F,
                    AOP.logical_shift_right, AOP.bitwise_and,
                )
                ul = mid.tile([P, 8, 2, 256], f16, tag="ul", name="ul")
                nc.vector.tensor_scalar(
                    ul[:, :rmx, 0, :],
                    ue.rearrange("p (r c) -> p r c", r=rmx),
                    0.25, None, AOP.mult,
                )
                nc.vector.tensor_scalar(
                    ul[:, :rmx, 1, :],
                    uh.rearrange("p (r c) -> p r c", r=rmx),
                    0.25, None, AOP.mult,
                )
                # u^2: even half on DVE (2x f16), odd half on ACT Square --
                # the two engines split what used to be the ACT Ln cost
                u2 = mid.tile([P, 8, 2, 256], f16, tag="u2", name="u2")
                if ci == 3:  # rebalance: ACT has slack early, DVE is pacer
                    nc.scalar.activation(
                        u2[:, :rmx, 0, :], ul[:, :rmx, 0, :], ACT.Square
                    )
                else:
                    nc.vector.tensor_tensor(
                        u2[:, :rmx, 0, :], ul[:, :rmx, 0, :],
                        ul[:, :rmx, 0, :], AOP.mult,
                    )
                nc.scalar.activation(
                    u2[:, :rmx, 1, :], ul[:, :rmx, 1, :], ACT.Square
                )
                lt = ul[:, :rmx, :, :].rearrange("p r a c -> p (r a c)")
                qt = u2[:, :rmx, :, :].rearrange("p r a c -> p (r a c)")

                for j, s in ((0, xt), (1, lt), (2, qt)):
                    if last_sub and j == 2:
                        break  # defer the last q MMs behind the xa x/l work
                    for rr in range(rm):
                        emit_mm(
                            r0 + rr,
                            j,
                            s,
                            slice(rr * NUM_SEG, (rr + 1) * NUM_SEG),
                        )

            # final round: host-supplied u (f16, parity-ordered), ACT-only
            # chain so the tail never waits on the DVE queue.  Endgame:
            # close the x and l chains and copy their PSUM banks while the
            # last q squares/MMs still run.
            lci, lr0, lrm = subops[-1]
            rl = rounds - 1
            qt2 = mid.tile([P, NUM_SEG], f16, tag="qtz", name="qt2")
            nc.scalar.activation(qt2[:], ulast[:], ACT.Square)
            xl = ctiles[-1][:, (rl - cstarts[-1]) * NUM_SEG :]
            emit_mm(rl, 0, xl, slice(0, NUM_SEG))
            emit_mm(rl, 1, ulast, slice(0, NUM_SEG))
            nc.scalar.activation(outsb[:, 0:2, :], acc01[:, :, :], ACT.Copy)
            for rr in range(lrm):
                emit_mm(lr0 + rr, 2, qt,
                        slice(rr * NUM_SEG, (rr + 1) * NUM_SEG))
            emit_mm(rl, 2, qt2, slice(0, NUM_SEG))
            nc.vector.tensor_copy(outsb[:, 2, :], acc2[:, :])
            nc.sync.dma_start(out_d[:], outsb[0:P:32, :, :])

    return nc


_PROG_CACHE = {}


def _get_prog(rounds=ROUNDS):
    if rounds not in _PROG_CACHE:
        nc = build_nc(rounds)
        fixed, _n = _split_sync_waits(nc.to_json_bytes())
        nc.to_json_bytes = lambda: fixed
        _PROG_CACHE[rounds] = nc
    return _PROG_CACHE[rounds]


def _finale(partials, target_mean, target_std):
    """partials: [512, 4] float64 summed across cores -> scalar loss."""
    xs = partials[:, 0]
    ls = partials[:, 1]
    qs = partials[:, 2]
    cnt = partials[:, 3]
    cg = np.maximum(cnt, 1.0)
    mean_w = xs / cg
    mean_log = ls / cg
    log_var = qs / cg - mean_log**2
    std_w = np.sqrt(log_var + EPS)
    mean_loss = np.mean((mean_w - target_mean.astype(np.float64)) ** 2)
    std_loss = np.mean((std_w - target_std.astype(np.float64)) ** 2)
    total = (1.0 - STD_WEIGHT) * mean_loss + STD_WEIGHT * std_loss
    return np.float32(total * STRENGTH)


def _bucketize(x, idx, rounds):
    """Group edges by segment into the padded per-core device layout."""
    import ml_dtypes

    cap = N_CORES * rounds * P
    counts = np.bincount(idx, minlength=NUM_SEG).astype(np.int64)
    order = np.argsort(idx, kind="stable")
    xs = np.asarray(x, dtype=np.float32)[order]
    offs = np.zeros(NUM_SEG + 1, dtype=np.int64)
    np.cumsum(counts, out=offs[1:])

    big = np.full((NUM_SEG, cap), 1.0, dtype=np.float32)
    for s in range(NUM_SEG):
        big[s, : counts[s]] = xs[offs[s] : offs[s + 1]]
    # [seg, core, round, part] -> per core [part, round, seg] flat
    a = big.reshape(NUM_SEG, N_CORES, rounds, P)
    in_maps = []
    for c in range(N_CORES):
        xc = np.ascontiguousarray(a[:, c].transpose(2, 1, 0)).reshape(
            P, rounds * NUM_SEG
        )
        x8 = xc.astype(ml_dtypes.float8_e5m2)
        # last round's u = e5m2 bits & 0x7f, parity-ordered [even|odd] to
        # match the on-device layout of the earlier rounds
        ub = ((x8[:, -NUM_SEG:].view(np.uint8) & 0x7F) / 4.0).astype(np.float16)
        ul = np.concatenate([ub[:, 0::2], ub[:, 1::2]], axis=1)
        in_maps.append({"x": x8, "ul": np.ascontiguousarray(ul)})
    return in_maps, counts


def run_partials(x, idx, trace=False):
    """Run the device program; return [512, 4] partials summed over cores."""
    _install_ntff_hook()
    from concourse.bass_utils import run_bass_kernel_spmd

    x = np.asarray(x, dtype=np.float32)
    idx = np.asarray(idx)

    rounds = ROUNDS
    max_cnt = int(np.bincount(idx, minlength=NUM_SEG).max())
    if max_cnt > N_CORES * rounds * P:  # pathological skew: grow capacity
        rounds = -(-max_cnt // (N_CORES * P)) + 1

    nc = _get_prog(rounds)
    in_maps, counts = _bucketize(x, idx, rounds)
    res = run_bass_kernel_spmd(nc, in_maps, list(range(N_CORES)), trace=trace)

    sums = np.zeros((3, NUM_SEG), dtype=np.float64)
    for c in range(N_CORES):
        o = res.results[c]["out"].astype(np.float64)  # [4, 3, 512]
        sums += o.sum(axis=0)
    pad = N_CORES * rounds * P - counts.astype(np.float64)
    cnt = counts.astype(np.float64)
    # l/q PSUM columns are parity-permuted: col i<256 -> seg 2i, else odd
    su = np.empty(NUM_SEG)
    su[0::2] = sums[1][: NUM_SEG // 2]
    su[1::2] = sums[1][NUM_SEG // 2 :]
    su2 = np.empty(NUM_SEG)
    su2[0::2] = sums[2][: NUM_SEG // 2]
    su2[1::2] = sums[2][NUM_SEG // 2 :]
    su *= 4.0      # device sums u/4
    su2 *= 16.0    # device sums (u/4)^2
    # pads are x = 1.0 -> u = 60, u^2 = 3600 (exact)
    su -= pad * 60.0
    su2 -= pad * 3600.0
    # Mitchell: ln|x| ~ k*u - c with k = ln2/4, c = 15*ln2 (exact algebra)
    k = np.log(2.0) / 4.0
    c_ = 15.0 * np.log(2.0)
    partials = np.zeros((NUM_SEG, 4), dtype=np.float64)
    partials[:, 0] = sums[0] - pad * 1.0          # pads are x = 1.0
    partials[:, 1] = k * su - c_ * cnt
    partials[:, 2] = k * k * su2 - 2 * k * c_ * su + c_ * c_ * cnt
    partials[:, 3] = counts
    return partials, res


def kernel(x, idx, target_mean, target_std):
    partials, _res = run_partials(x, idx, trace=False)
    return _finale(
        partials, np.asarray(target_mean), np.asarray(target_std)
    )

